# revision 21
# baseline (speedup 1.0000x reference)
"""AdjacencyAwareMultiHeadAttention on 8 trn2 NeuronCores.

Sharding: data-parallel over batch. Core b handles graph b entirely
(all 8 heads). Host does input repacking (transposes / dtype prep),
final normalization by the softmax denominator, and the 4 scalar
diagnostic means.

Device math per core (graph), S^T layout ([m=key on partitions, n=query free]):
  QT/KT = WT.T @ hT (+bias)            [256, 1024] f32 (two 128-row blocks)
  V     = hT.T @ WvT (+bias)           [1024, 264] with a ones column per head
  per (head, n-chunk, m-tile):
    S^T   = KT_h_slice.T @ QT_h_slice          (PSUM, f32)
    P     = exp(S * 1/sqrt(D) + keypad_bias)   (ScalarE, -> bf16 SBUF)
    P'    = P * alpha^A                        (VectorE, bf16)
    T0,T1 = P' * (A==0), P' * (A==1)           (VectorE, bf16)
    O'   += [V_h | 1].T @ P'   (rows 0-31: unnormalized out^T, row 32: colsum P')
    Z    += ones.T @ P         (softmax denominator)
    U0   += ones.T @ T0 ; U1 += ones.T @ T1
Host: O = O'[0:32]/Z, diagnostics from Z/U0/U1/colsumP' and exact mask
counts of A.
"""

import math
import os
import sys

import numpy as np

sys.path.insert(0, "/opt/trn_rl_repo")

import concourse.bass as bass
import concourse.mybir as mybir
from concourse.tile import TileContext
from concourse import bacc, bass_utils

B, N, IND, D, H = 8, 1024, 128, 32, 8
HD = H * D  # 256
NT = N // 128  # 8 m-tiles
NCH = N // 512  # 2 n-chunks
SCALE = 1.0 / math.sqrt(D)
NEG = -60.0  # key-pad bias: exp(-60) == 0 for our logit range

F32 = mybir.dt.float32
BF16 = mybir.dt.bfloat16

_last_exec_time_ns = None


# column offsets inside the packed [128, IN_COLS] input
OFF_HT = 0
OFF_A = 1024
OFF_KB = OFF_A + NT * N          # 9216
OFF_WQ = OFF_KB + NT             # 9224
OFF_WK = OFF_WQ + HD             # 9480
OFF_WV = OFF_WK + HD             # 9736
OFF_BQ = OFF_WV + HD             # 9992
OFF_BK = OFF_BQ + 2              # 9994
OFF_BV = OFF_BK + 2              # 9996
IN_COLS = OFF_BV + HD            # 10252


def _build(ln_alpha: float) -> bass.Bass:
    nc = bacc.Bacc()

    in_d = nc.dram_tensor("IN", [128, IN_COLS], F32, kind="ExternalInput")

    O_d = nc.dram_tensor("O", [H, 33, N], F32, kind="ExternalOutput")
    Zu_d = nc.dram_tensor("Zu", [H, 3, N], F32, kind="ExternalOutput")

    with TileContext(nc) as tc:
        with (
            tc.tile_pool(name="const", bufs=1) as cpool,
            tc.tile_pool(name="stage", bufs=3) as spool,
            tc.tile_pool(name="ppool", bufs=3) as ppool,
            tc.tile_pool(name="psum_big", bufs=3, space="PSUM") as ps_big,
            tc.tile_pool(name="psum_o", bufs=2, space="PSUM") as ps_o,
            tc.tile_pool(name="psum_row", bufs=3, space="PSUM") as ps_row,
        ):
            # ---- load all inputs in one DMA ----
            IN = cpool.tile([128, IN_COLS], F32, tag="IN")
            nc.sync.dma_start(IN[:], in_d[:])
            hT = IN[:, OFF_HT:OFF_HT + N]
            Af = IN[:, OFF_A:OFF_A + NT * N]
            kb = IN[:, OFF_KB:OFF_KB + NT]
            wqT = IN[:, OFF_WQ:OFF_WQ + HD]
            wkT = IN[:, OFF_WK:OFF_WK + HD]
            wvT = IN[:, OFF_WV:OFF_WV + HD]
            bqT = IN[:, OFF_BQ:OFF_BQ + 2]
            bkT = IN[:, OFF_BK:OFF_BK + 2]
            bvR = IN[:, OFF_BV:OFF_BV + HD]

            ones = cpool.tile([128, 1], BF16, tag="ones")
            nc.vector.memset(ones[:], 1.0)

            # ---- A-derived tiles: W = alpha^A, M0 = (A==0), M1 = (A==1) ----
            Wsb = cpool.tile([128, NT * N], BF16, tag="Wsb")
            M0 = cpool.tile([128, NT * N], BF16, tag="M0")
            M1 = cpool.tile([128, NT * N], BF16, tag="M1")
            for mi in range(NT):
                for ch in range(NCH):
                    sl = bass.ds(mi * N + ch * 512, 512)
                    nc.scalar.activation(
                        Wsb[:, sl], Af[:, sl],
                        mybir.ActivationFunctionType.Exp, scale=ln_alpha,
                    )
                    nc.vector.tensor_scalar(
                        out=M0[:, sl], in0=Af[:, sl], scalar1=0.0, scalar2=None,
                        op0=mybir.AluOpType.is_equal,
                    )
                    nc.vector.tensor_scalar(
                        out=M1[:, sl], in0=Af[:, sl], scalar1=1.0, scalar2=None,
                        op0=mybir.AluOpType.is_equal,
                    )

            # ---- projections ----
            # QT/KT: [2][128, 1024] f32, block c = heads 4c..4c+3
            QT = [cpool.tile([128, N], F32, tag=f"QT{c}", name=f"QT{c}")
                  for c in range(2)]
            KT = [cpool.tile([128, N], F32, tag=f"KT{c}", name=f"KT{c}")
                  for c in range(2)]
            for c in range(2):
                for ch in range(NCH):
                    nsl = bass.ds(ch * 512, 512)
                    csl = bass.ds(c * 128, 128)
                    pq = ps_big.tile([128, 512], F32, tag="bigp")
                    nc.tensor.matmul(pq[:], wqT[:, csl], hT[:, nsl],
                                     start=True, stop=True)
                    nc.vector.tensor_scalar_add(QT[c][:, nsl], pq[:], bqT[:, c:c + 1])
                    pk = ps_big.tile([128, 512], F32, tag="bigp")
                    nc.tensor.matmul(pk[:], wkT[:, csl], hT[:, nsl],
                                     start=True, stop=True)
                    nc.vector.tensor_scalar_add(KT[c][:, nsl], pk[:], bkT[:, c:c + 1])

            # V: Vsb[p, mi*256 + h*32 + d]
            Vsb = cpool.tile([128, NT * HD], BF16, tag="Vsb")
            for mi in range(NT):
                pv = ps_big.tile([128, HD], F32, tag="bigp")
                nc.tensor.matmul(pv[:], hT[:, bass.ds(mi * 128, 128)], wvT[:],
                                 start=True, stop=True)
                nc.vector.tensor_add(
                    Vsb[:, bass.ds(mi * HD, HD)], pv[:], bvR[:])

            # ---- main attention loops: 2-head blocks ----
            # rowsA: Z(h0)@0, Z(h1)@32, U0(h0)@64, U0(h1)@96
            # rowsB: U1(h0)@0, U1(h1)@32, F(h0)@64, F(h1)@96
            for blk in range(H // 2):
                h0 = 2 * blk
                c = h0 // 4
                for ch in range(NCH):
                    nsl = bass.ds(ch * 512, 512)
                    Op = ps_o.tile([64, 512], F32, tag="Op")
                    rowsA = ps_row.tile([97, 512], F32, tag="rows", name="rA")
                    rowsB = ps_row.tile([97, 512], F32, tag="rows", name="rB")
                    for mi in range(NT):
                        first, last = mi == 0, mi == NT - 1
                        asl = bass.ds(mi * N + ch * 512, 512)
                        Ps, Pps = [], []
                        for side in range(2):
                            h = h0 + side
                            r = h % 4
                            Sp = ps_big.tile([128, 512], F32, tag="bigp")
                            nc.tensor.matmul(
                                Sp[:],
                                KT[c][bass.ds(r * 32, 32),
                                      bass.ds(mi * 128, 128)],
                                QT[c][bass.ds(r * 32, 32), nsl],
                                start=True, stop=True,
                                tile_position=(r * 32, 0),
                            )
                            P = ppool.tile([128, 512], BF16, tag=f"P{side}")
                            nc.scalar.activation(
                                P[:], Sp[:], mybir.ActivationFunctionType.Exp,
                                bias=kb[:, mi:mi + 1], scale=SCALE,
                            )
                            Pp = ppool.tile([128, 512], BF16, tag=f"Pp{side}")
                            nc.vector.tensor_mul(Pp[:], P[:], Wsb[:, asl])
                            Ps.append(P)
                            Pps.append(Pp)
                        for side in range(2):
                            h = h0 + side
                            P, Pp = Ps[side], Pps[side]
                            T0 = ppool.tile([128, 512], BF16, tag=f"T0{side}")
                            nc.vector.tensor_mul(T0[:], Pp[:], M0[:, asl])
                            T1 = ppool.tile([128, 512], BF16, tag=f"T1{side}")
                            nc.gpsimd.tensor_mul(T1[:], Pp[:], M1[:, asl])
                            o = 32 * side
                            nc.tensor.matmul(
                                Op[o:o + 32, :],
                                Vsb[:, bass.ds(mi * HD + h * D, D)], Pp[:],
                                start=first, stop=last,
                                tile_position=(0, o))
                            nc.tensor.matmul(
                                rowsA[o:o + 1, :], ones[:], P[:],
                                start=first, stop=last, tile_position=(0, o))
                            nc.tensor.matmul(
                                rowsA[64 + o:65 + o, :], ones[:], T0[:],
                                start=first, stop=last,
                                tile_position=(0, 64 + o))
                            nc.tensor.matmul(
                                rowsB[o:o + 1, :], ones[:], T1[:],
                                start=first, stop=last, tile_position=(0, o))
                            nc.tensor.matmul(
                                rowsB[64 + o:65 + o, :], ones[:], Pp[:],
                                start=first, stop=last,
                                tile_position=(0, 64 + o))
                    Ostg = spool.tile([64, 512], F32, tag="Ostg")
                    nc.any.tensor_copy(Ostg[:], Op[:])
                    rstgA = spool.tile([97, 512], F32, tag="rstgA")
                    nc.any.tensor_copy(rstgA[:], rowsA[:])
                    rstgB = spool.tile([97, 512], F32, tag="rstgB")
                    nc.any.tensor_copy(rstgB[:], rowsB[:])
                    for side in range(2):
                        h = h0 + side
                        o = 32 * side
                        nc.sync.dma_start(O_d[h, 0:32, nsl],
                                          Ostg[o:o + 32, :])
                        nc.sync.dma_start(Zu_d[h, 0:1, nsl],
                                          rstgA[o:o + 1, :])
                        nc.sync.dma_start(Zu_d[h, 1:2, nsl],
                                          rstgA[64 + o:65 + o, :])
                        nc.sync.dma_start(Zu_d[h, 2:3, nsl],
                                          rstgB[o:o + 1, :])
                        nc.sync.dma_start(O_d[h, 32:33, nsl],
                                          rstgB[64 + o:65 + o, :])
    return nc


def kernel(h, A, lengths, alpha, Wq, bq, Wk, bk, Wv, bv):
    global _last_exec_time_ns
    h = np.asarray(h, np.float32)
    A = np.asarray(A)
    lengths = np.asarray(lengths)
    alpha_v = float(np.asarray(alpha).reshape(-1)[0])
    ln_alpha = math.log(alpha_v)

    wqT = np.ascontiguousarray(np.asarray(Wq, np.float32).T)  # [128, 256]
    wkT = np.ascontiguousarray(np.asarray(Wk, np.float32).T)
    wvT = np.ascontiguousarray(np.asarray(Wv, np.float32).T)
    bqT = np.ascontiguousarray(np.asarray(bq, np.float32).reshape(2, 128).T)
    bkT = np.ascontiguousarray(np.asarray(bk, np.float32).reshape(2, 128).T)
    bvR = np.tile(np.asarray(bv, np.float32)[None, :], (128, 1))

    nc = _build(ln_alpha)
    nc.finalize()

    in_maps = []
    for b in range(B):
        IN = np.empty((128, IN_COLS), np.float32)
        IN[:, OFF_HT:OFF_HT + N] = h[b].T
        # Af[p, mi*N + n] = A[b, n, mi*128+p]  (att is reweighted by
        # alpha^A[query, key]; our tiles are [key, query])
        IN[:, OFF_A:OFF_A + NT * N] = (
            np.ascontiguousarray(A[b].T).astype(np.float32)
            .reshape(NT, 128, N).transpose(1, 0, 2).reshape(128, NT * N))
        kbv = np.where(np.arange(N) < int(lengths[b]), 0.0, NEG)
        IN[:, OFF_KB:OFF_KB + NT] = kbv.reshape(NT, 128).T
        IN[:, OFF_WQ:OFF_WQ + HD] = wqT
        IN[:, OFF_WK:OFF_WK + HD] = wkT
        IN[:, OFF_WV:OFF_WV + HD] = wvT
        IN[:, OFF_BQ:OFF_BQ + 2] = bqT
        IN[:, OFF_BK:OFF_BK + 2] = bkT
        IN[:, OFF_BV:OFF_BV + HD] = bvR
        in_maps.append({"IN": IN})

    trace = bool(os.environ.get("KERNEL_TRACE"))
    try:
        res = bass_utils.run_bass_kernel_spmd(
            nc, in_maps, core_ids=list(range(B)), trace=trace)
    except ModuleNotFoundError:
        res = bass_utils.run_bass_kernel_spmd(
            nc, in_maps, core_ids=list(range(B)), trace=False)
    _last_exec_time_ns = getattr(res, "exec_time_ns", None)
    if _last_exec_time_ns is not None:
        print(f"HW exec time: {_last_exec_time_ns} ns")
    outs = res.results
    global _last_outs
    _last_outs = outs

    # ---- host-side gather / normalize / diagnostics ----
    h_heads = np.zeros((B, H, N, D), np.float32)
    U0 = U1 = F = 0.0
    for b in range(B):
        O = np.asarray(outs[b]["O"], np.float32)      # [H, 33, N]
        Zu = np.asarray(outs[b]["Zu"], np.float32)    # [H, 3, N]
        ln = int(lengths[b])
        Z = Zu[:, 0, :]                               # [H, N]
        rz = np.zeros_like(Z)
        rz[:, :ln] = 1.0 / Z[:, :ln]
        h_heads[b] = (O[:, 0:D, :] * rz[:, None, :]).transpose(0, 2, 1)
        F += float((O[:, D, :] * rz).sum())
        U0 += float((Zu[:, 1, :] * rz).sum())
        U1 += float((Zu[:, 2, :] * rz).sum())

    cnt1 = float(H) * float(np.count_nonzero(A == 1))
    cnt2 = float(H) * float(np.count_nonzero(A > 1))
    S_tot = float(H) * float(np.sum(lengths))
    S1 = U1 / alpha_v
    pre_d1 = S1 / cnt1
    pre_d2 = (S_tot - U0 - S1) / cnt2
    post_d1 = U1 / cnt1
    post_d2 = (F - U0 - U1) / cnt2
    return (h_heads, np.float32(pre_d1), np.float32(pre_d2),
            np.float32(post_d1), np.float32(post_d2))


# revision 23
# speedup vs baseline: 1.5979x; 1.5979x over previous
"""AdjacencyAwareMultiHeadAttention on 8 trn2 NeuronCores.

Sharding: data-parallel over batch. Core b handles graph b entirely
(all 8 heads). Host does input repacking (transposes / dtype prep),
final normalization by the softmax denominator, and the 4 scalar
diagnostic means.

Device math per core (graph), S^T layout ([m=key on partitions, n=query free]):
  QT/KT = WT.T @ hT (+bias)            [256, 1024] f32 (two 128-row blocks)
  V     = hT.T @ WvT (+bias)           [1024, 264] with a ones column per head
  per (head, n-chunk, m-tile):
    S^T   = KT_h_slice.T @ QT_h_slice          (PSUM, f32)
    P     = exp(S * 1/sqrt(D) + keypad_bias)   (ScalarE, -> bf16 SBUF)
    P'    = P * alpha^A                        (VectorE, bf16)
    T0,T1 = P' * (A==0), P' * (A==1)           (VectorE, bf16)
    O'   += [V_h | 1].T @ P'   (rows 0-31: unnormalized out^T, row 32: colsum P')
    Z    += ones.T @ P         (softmax denominator)
    U0   += ones.T @ T0 ; U1 += ones.T @ T1
Host: O = O'[0:32]/Z, diagnostics from Z/U0/U1/colsumP' and exact mask
counts of A.
"""

import math
import os
import sys

import numpy as np

sys.path.insert(0, "/opt/trn_rl_repo")

import concourse.bass as bass
import concourse.mybir as mybir
from concourse.tile import TileContext
from concourse import bacc, bass_utils

B, N, IND, D, H = 8, 1024, 128, 32, 8
HD = H * D  # 256
NT = N // 128  # 8 m-tiles
NCH = N // 512  # 2 n-chunks
SCALE = 1.0 / math.sqrt(D)
NEG = -60.0  # key-pad bias: exp(-60) == 0 for our logit range

F32 = mybir.dt.float32
BF16 = mybir.dt.bfloat16

_last_exec_time_ns = None


# column offsets inside the packed [128, IN_COLS] input
OFF_HT = 0
OFF_A = 1024
OFF_KB = OFF_A + NT * N          # 9216
OFF_WQ = OFF_KB + NT             # 9224
OFF_WK = OFF_WQ + HD             # 9480
OFF_WV = OFF_WK + HD             # 9736
OFF_BQ = OFF_WV + HD             # 9992
OFF_BK = OFF_BQ + 2              # 9994
OFF_BV = OFF_BK + 2              # 9996
IN_COLS = OFF_BV + HD            # 10252


def _build(ln_alpha: float) -> bass.Bass:
    nc = bacc.Bacc()

    in_d = nc.dram_tensor("IN", [128, IN_COLS], F32, kind="ExternalInput")

    O_d = nc.dram_tensor("O", [H, 33, N], F32, kind="ExternalOutput")
    Zu_d = nc.dram_tensor("Zu", [H, 3, N], F32, kind="ExternalOutput")

    with TileContext(nc) as tc:
        with (
            tc.tile_pool(name="const", bufs=1) as cpool,
            tc.tile_pool(name="stage", bufs=3) as spool,
            tc.tile_pool(name="ppool", bufs=3) as ppool,
            tc.tile_pool(name="psum_big", bufs=3, space="PSUM") as ps_big,
            tc.tile_pool(name="psum_o", bufs=2, space="PSUM") as ps_o,
            tc.tile_pool(name="psum_row", bufs=3, space="PSUM") as ps_row,
        ):
            # ---- load all inputs in one DMA ----
            IN = cpool.tile([128, IN_COLS], F32, tag="IN")
            nc.sync.dma_start(IN[:], in_d[:])
            hT = IN[:, OFF_HT:OFF_HT + N]
            Af = IN[:, OFF_A:OFF_A + NT * N]
            kb = IN[:, OFF_KB:OFF_KB + NT]
            wqT = IN[:, OFF_WQ:OFF_WQ + HD]
            wkT = IN[:, OFF_WK:OFF_WK + HD]
            wvT = IN[:, OFF_WV:OFF_WV + HD]
            bqT = IN[:, OFF_BQ:OFF_BQ + 2]
            bkT = IN[:, OFF_BK:OFF_BK + 2]
            bvR = IN[:, OFF_BV:OFF_BV + HD]

            ones = cpool.tile([128, 1], BF16, tag="ones")
            nc.vector.memset(ones[:], 1.0)

            # ---- A-derived tiles: W = alpha^A, M0 = (A==0), M1 = (A==1) ----
            Wsb = cpool.tile([128, NT * N], BF16, tag="Wsb")
            M0 = cpool.tile([128, NT * N], BF16, tag="M0")
            M1 = cpool.tile([128, NT * N], BF16, tag="M1")
            for mi in range(NT):
                for ch in range(NCH):
                    sl = bass.ds(mi * N + ch * 512, 512)
                    nc.scalar.activation(
                        Wsb[:, sl], Af[:, sl],
                        mybir.ActivationFunctionType.Exp, scale=ln_alpha,
                    )
                    nc.vector.tensor_scalar(
                        out=M0[:, sl], in0=Af[:, sl], scalar1=0.0, scalar2=None,
                        op0=mybir.AluOpType.is_equal,
                    )
                    nc.vector.tensor_scalar(
                        out=M1[:, sl], in0=Af[:, sl], scalar1=1.0, scalar2=None,
                        op0=mybir.AluOpType.is_equal,
                    )

            # ---- projections ----
            # QT/KT: [2][128, 1024] f32, block c = heads 4c..4c+3
            QT = [cpool.tile([128, N], F32, tag=f"QT{c}", name=f"QT{c}")
                  for c in range(2)]
            KT = [cpool.tile([128, N], F32, tag=f"KT{c}", name=f"KT{c}")
                  for c in range(2)]
            for c in range(2):
                for ch in range(NCH):
                    nsl = bass.ds(ch * 512, 512)
                    csl = bass.ds(c * 128, 128)
                    pq = ps_big.tile([128, 512], F32, tag="bigp")
                    nc.tensor.matmul(pq[:], wqT[:, csl], hT[:, nsl],
                                     start=True, stop=True)
                    nc.vector.tensor_scalar_add(QT[c][:, nsl], pq[:], bqT[:, c:c + 1])
                    pk = ps_big.tile([128, 512], F32, tag="bigp")
                    nc.tensor.matmul(pk[:], wkT[:, csl], hT[:, nsl],
                                     start=True, stop=True)
                    nc.vector.tensor_scalar_add(KT[c][:, nsl], pk[:], bkT[:, c:c + 1])

            # V with ones column: Vp1[p, mi*264 + h*33 + d], d=32 -> 1.0
            Vp1 = cpool.tile([128, NT * 264], BF16, tag="Vp1")
            nc.vector.memset(Vp1[:], 1.0)
            for mi in range(NT):
                pv = ps_big.tile([128, HD], F32, tag="bigp")
                nc.tensor.matmul(pv[:], hT[:, bass.ds(mi * 128, 128)], wvT[:],
                                 start=True, stop=True)
                vtmp = ps_big.tile([128, HD], F32, tag="bigp")
                nc.vector.tensor_add(vtmp[:], pv[:], bvR[:])
                dst = Vp1[:, bass.ds(mi * 264, 264)].rearrange(
                    "p (h x) -> p h x", x=33)[:, :, 0:D]
                nc.vector.tensor_copy(
                    dst, vtmp[:].rearrange("p (h d) -> p h d", d=D))

            # ---- main attention loops: 2-head blocks ----
            # Op: head0 [V|1]-out at partitions 0..32, head1 at 64..96
            # rowsA: Z(h0)@0, Z(h1)@32, U0(h0)@64, U0(h1)@96
            # rowsB: U1(h0)@0, U1(h1)@32
            for blk in range(H // 2):
                h0 = 2 * blk
                c = h0 // 4
                for ch in range(NCH):
                    nsl = bass.ds(ch * 512, 512)
                    Op = ps_o.tile([97, 512], F32, tag="Op")
                    rowsA = ps_row.tile([97, 512], F32, tag="rows", name="rA")
                    rowsB = ps_row.tile([33, 512], F32, tag="rows", name="rB")
                    for mi in range(NT):
                        first, last = mi == 0, mi == NT - 1
                        asl = bass.ds(mi * N + ch * 512, 512)
                        Ps, Pps = [], []
                        for side in range(2):
                            h = h0 + side
                            r = h % 4
                            Sp = ps_big.tile([128, 512], F32, tag="bigp")
                            nc.tensor.matmul(
                                Sp[:],
                                KT[c][bass.ds(r * 32, 32),
                                      bass.ds(mi * 128, 128)],
                                QT[c][bass.ds(r * 32, 32), nsl],
                                start=True, stop=True,
                                tile_position=(r * 32, 0),
                            )
                            P = ppool.tile([128, 512], BF16, tag=f"P{side}")
                            nc.scalar.activation(
                                P[:], Sp[:], mybir.ActivationFunctionType.Exp,
                                bias=kb[:, mi:mi + 1], scale=SCALE,
                            )
                            Pp = ppool.tile([128, 512], BF16, tag=f"Pp{side}")
                            nc.vector.tensor_mul(Pp[:], P[:], Wsb[:, asl])
                            Ps.append(P)
                            Pps.append(Pp)
                        for side in range(2):
                            h = h0 + side
                            P, Pp = Ps[side], Pps[side]
                            T0 = ppool.tile([128, 512], BF16, tag=f"T0{side}")
                            nc.any.tensor_tensor(
                                out=T0[:], in0=Pp[:], in1=M0[:, asl],
                                op=mybir.AluOpType.mult)
                            T1 = ppool.tile([128, 512], BF16, tag=f"T1{side}")
                            nc.any.tensor_tensor(
                                out=T1[:], in0=Pp[:], in1=M1[:, asl],
                                op=mybir.AluOpType.mult)
                            o = 64 * side
                            nc.tensor.matmul(
                                Op[o:o + 33, :],
                                Vp1[:, bass.ds(mi * 264 + h * 33, 33)], Pp[:],
                                start=first, stop=last,
                                tile_position=(0, o))
                            q = 32 * side
                            nc.tensor.matmul(
                                rowsA[q:q + 1, :], ones[:], P[:],
                                start=first, stop=last, tile_position=(0, q))
                            nc.tensor.matmul(
                                rowsA[64 + q:65 + q, :], ones[:], T0[:],
                                start=first, stop=last,
                                tile_position=(0, 64 + q))
                            nc.tensor.matmul(
                                rowsB[q:q + 1, :], ones[:], T1[:],
                                start=first, stop=last, tile_position=(0, q))
                    Ostg = spool.tile([97, 512], F32, tag="Ostg")
                    nc.any.tensor_copy(Ostg[:], Op[:])
                    rstgA = spool.tile([97, 512], F32, tag="rstgA")
                    nc.any.tensor_copy(rstgA[:], rowsA[:])
                    rstgB = spool.tile([33, 512], F32, tag="rstgB")
                    nc.any.tensor_copy(rstgB[:], rowsB[:])
                    for side in range(2):
                        h = h0 + side
                        o = 64 * side
                        q = 32 * side
                        nc.sync.dma_start(O_d[h, 0:33, nsl],
                                          Ostg[o:o + 33, :])
                        nc.sync.dma_start(Zu_d[h, 0:1, nsl],
                                          rstgA[q:q + 1, :])
                        nc.sync.dma_start(Zu_d[h, 1:2, nsl],
                                          rstgA[64 + q:65 + q, :])
                        nc.sync.dma_start(Zu_d[h, 2:3, nsl],
                                          rstgB[q:q + 1, :])
    return nc


def kernel(h, A, lengths, alpha, Wq, bq, Wk, bk, Wv, bv):
    global _last_exec_time_ns
    h = np.asarray(h, np.float32)
    A = np.asarray(A)
    lengths = np.asarray(lengths)
    alpha_v = float(np.asarray(alpha).reshape(-1)[0])
    ln_alpha = math.log(alpha_v)

    wqT = np.ascontiguousarray(np.asarray(Wq, np.float32).T)  # [128, 256]
    wkT = np.ascontiguousarray(np.asarray(Wk, np.float32).T)
    wvT = np.ascontiguousarray(np.asarray(Wv, np.float32).T)
    bqT = np.ascontiguousarray(np.asarray(bq, np.float32).reshape(2, 128).T)
    bkT = np.ascontiguousarray(np.asarray(bk, np.float32).reshape(2, 128).T)
    bvR = np.tile(np.asarray(bv, np.float32)[None, :], (128, 1))

    nc = _build(ln_alpha)
    nc.finalize()

    in_maps = []
    for b in range(B):
        IN = np.empty((128, IN_COLS), np.float32)
        IN[:, OFF_HT:OFF_HT + N] = h[b].T
        # Af[p, mi*N + n] = A[b, n, mi*128+p]  (att is reweighted by
        # alpha^A[query, key]; our tiles are [key, query])
        IN[:, OFF_A:OFF_A + NT * N] = (
            np.ascontiguousarray(A[b].T).astype(np.float32)
            .reshape(NT, 128, N).transpose(1, 0, 2).reshape(128, NT * N))
        kbv = np.where(np.arange(N) < int(lengths[b]), 0.0, NEG)
        IN[:, OFF_KB:OFF_KB + NT] = kbv.reshape(NT, 128).T
        IN[:, OFF_WQ:OFF_WQ + HD] = wqT
        IN[:, OFF_WK:OFF_WK + HD] = wkT
        IN[:, OFF_WV:OFF_WV + HD] = wvT
        IN[:, OFF_BQ:OFF_BQ + 2] = bqT
        IN[:, OFF_BK:OFF_BK + 2] = bkT
        IN[:, OFF_BV:OFF_BV + HD] = bvR
        in_maps.append({"IN": IN})

    trace = bool(os.environ.get("KERNEL_TRACE"))
    try:
        res = bass_utils.run_bass_kernel_spmd(
            nc, in_maps, core_ids=list(range(B)), trace=trace)
    except ModuleNotFoundError:
        res = bass_utils.run_bass_kernel_spmd(
            nc, in_maps, core_ids=list(range(B)), trace=False)
    _last_exec_time_ns = getattr(res, "exec_time_ns", None)
    if _last_exec_time_ns is not None:
        print(f"HW exec time: {_last_exec_time_ns} ns")
    outs = res.results
    global _last_outs
    _last_outs = outs

    # ---- host-side gather / normalize / diagnostics ----
    h_heads = np.zeros((B, H, N, D), np.float32)
    U0 = U1 = F = 0.0
    for b in range(B):
        O = np.asarray(outs[b]["O"], np.float32)      # [H, 33, N]
        Zu = np.asarray(outs[b]["Zu"], np.float32)    # [H, 3, N]
        ln = int(lengths[b])
        Z = Zu[:, 0, :]                               # [H, N]
        rz = np.zeros_like(Z)
        rz[:, :ln] = 1.0 / Z[:, :ln]
        h_heads[b] = (O[:, 0:D, :] * rz[:, None, :]).transpose(0, 2, 1)
        F += float((O[:, D, :] * rz).sum())
        U0 += float((Zu[:, 1, :] * rz).sum())
        U1 += float((Zu[:, 2, :] * rz).sum())

    cnt1 = float(H) * float(np.count_nonzero(A == 1))
    cnt2 = float(H) * float(np.count_nonzero(A > 1))
    S_tot = float(H) * float(np.sum(lengths))
    S1 = U1 / alpha_v
    pre_d1 = S1 / cnt1
    pre_d2 = (S_tot - U0 - S1) / cnt2
    post_d1 = U1 / cnt1
    post_d2 = (F - U0 - U1) / cnt2
    return (h_heads, np.float32(pre_d1), np.float32(pre_d2),
            np.float32(post_d1), np.float32(post_d2))
